# revision 8
# baseline (speedup 1.0000x reference)
"""Trainium2 Bass kernel for spatial self-attention (nn_Attention_695784702726).

Reference computes, for x [4, 128, 64, 64]:
  qkv = w_qkv @ x (1x1 conv), heads=4, dim_head=32, n=4096 tokens
  per (b, head): S = (q^T k) * scale; P = softmax_j(S); O = P @ v^T
  out = w_out @ concat_heads(O) + b_out

Sharding: 16 (b, head) pairs over 8 cores -> core c owns batch b=c//2 and
head group g=c%2 (heads 2g, 2g+1). Each core computes a partial output
projection w_out[:, 64g:64g+64] @ O_heads; host sums the two partials per
batch and adds b_out. No collectives needed.

Per-core kernel layout ("S^T formulation" - softmax runs with j on the
partition axis so the PV matmul needs no transposes of P):
  S^T[j, i] = k[d, j]^T q[d, i] : lhsT = k slice (K=d=32), rhs = q slice.
  K=32 underutilizes the 128-deep PE array 4x, so 3 j-tiles are packed
  concurrently via PE row-tiling (tile_position row strips 0/32/64, derived
  automatically from base partitions of 3x-replicated q/k buffers).
  exp: one ScalarE activation per 3-bank PSUM group (FD=1536) -> P^T in SBUF.
  PV: out[d0..32 | rowsum, i] accumulates over j-tiles with lhsT = [v^T | 1].
  Normalize by reciprocal(rowsum) broadcast via a K=1 matmul.
"""

import numpy as np

HEADS = 4
DIM_HEAD = 32
SCALE = DIM_HEAD ** -0.5
B, C, H, W = 4, 128, 64, 64
N = H * W  # 4096 tokens
NCORES = 8
REP = 3  # PE row-strips packed per QK group

_CACHE = {}


def _build_bass():
    import concourse.bass as bass
    from concourse import bacc
    import concourse.mybir as mybir
    import concourse.tile as tile
    from concourse.masks import make_identity

    f32 = mybir.dt.float32
    nc = bacc.Bacc("TRN2", target_bir_lowering=False)

    x_ext = nc.declare_dram_parameter("x", [C, N], f32, isOutput=False)
    wq_ext = nc.declare_dram_parameter("wq", [C, 2 * 32 * REP], f32, isOutput=False)
    wk_ext = nc.declare_dram_parameter("wk", [C, 2 * 32 * REP], f32, isOutput=False)
    wv_ext = nc.declare_dram_parameter("wv", [C, 64], f32, isOutput=False)
    wo_ext = nc.declare_dram_parameter("wo", [64, C], f32, isOutput=False)
    out_ext = nc.declare_dram_parameter("out", [C, N], f32, isOutput=True)

    NCH = N // 512  # 512-wide column chunks
    JT = N // 128   # 32 j-tiles of 128
    # j-tile groups of REP processed concurrently in PE row strips
    groups = [list(range(s, min(s + REP, JT))) for s in range(0, JT, REP)]

    with tile.TileContext(nc) as tc:
        with tc.tile_pool(name="wpool", bufs=1) as wpool, \
             tc.tile_pool(name="qkv", bufs=1) as qkv_pool, \
             tc.tile_pool(name="outp", bufs=1) as out_pool:
            wq_sb = wpool.tile([C, 2 * 32 * REP], f32, tag="wq")
            wk_sb = wpool.tile([C, 2 * 32 * REP], f32, tag="wk")
            wv_sb = wpool.tile([C, 64], f32, tag="wv")
            wo_sb = wpool.tile([64, C], f32, tag="wo")
            ident = wpool.tile([32, 32], f32, tag="ident")
            ones1 = wpool.tile([1, 32], f32, tag="ones1")
            nc.gpsimd.dma_start(wq_sb[:, :], wq_ext[:, :])
            nc.gpsimd.dma_start(wk_sb[:, :], wk_ext[:, :])
            nc.gpsimd.dma_start(wv_sb[:, :], wv_ext[:, :])
            nc.gpsimd.dma_start(wo_sb[:, :], wo_ext[:, :])
            make_identity(nc, ident[:, :])
            nc.vector.memset(ones1[:, :], 1.0)

            # Per-head replicated q/k ([32*REP, N]) so each PE row strip has
            # its own copy of the K=32 operands; v stays unreplicated.
            q_rep = [qkv_pool.tile([32 * REP, N], f32, tag=f"q{h}", name=f"qrep{h}") for h in range(2)]
            k_rep = [qkv_pool.tile([32 * REP, N], f32, tag=f"k{h}", name=f"krep{h}") for h in range(2)]
            v_sb = [qkv_pool.tile([32, N], f32, tag=f"v{h}", name=f"vsb{h}") for h in range(2)]
            # v^T with a ones column appended per j-tile: [128, 33] per tile.
            vt1 = [qkv_pool.tile([128, 33 * JT], f32, tag=f"vt{h}", name=f"vt1{h}") for h in range(2)]
            o_sb = out_pool.tile([64, N], f32, tag="o")
            out_sb = out_pool.tile([C, N], f32, tag="outsb")

            with tc.tile_pool(name="xpool", bufs=1) as xpool, \
                 tc.tile_pool(name="ppsum", bufs=2, space="PSUM") as ppsum, \
                 tc.tile_pool(name="tpsum", bufs=2, space="PSUM") as tpsum:
                x_sb = xpool.tile([C, N], f32, tag="x")
                nc.gpsimd.dma_start(x_sb[:, :], x_ext[:, :])

                for h in range(2):
                    nc.vector.memset(vt1[h][:, :], 1.0)

                # PE wait-slot workaround: LDWEIGHTS takes at most one
                # embedded semaphore wait, so pre-absorb each producer's
                # semaphore into the PE vector clock with 1-dep dummy matmuls
                # before any real matmul needs two of them at once.
                for tsrc in [wq_sb, wk_sb, wv_sb, wo_sb, ident, x_sb]:
                    nc.tensor.ldweights(
                        tsrc[0:32, 0:1].bitcast(mybir.dt.bfloat16))

                for ch in range(NCH):
                    cs = slice(512 * ch, 512 * (ch + 1))
                    for h in range(2):
                        pq = ppsum.tile([128, 512], f32, tag="proj")
                        nc.tensor.matmul(pq[0:32 * REP, :],
                                         wq_sb[:, 96 * h:96 * h + 96],
                                         x_sb[:, cs], start=True, stop=True)
                        nc.vector.tensor_copy(q_rep[h][:, cs], pq[0:32 * REP, :])
                        pk = ppsum.tile([128, 512], f32, tag="proj")
                        nc.tensor.matmul(pk[0:32 * REP, :],
                                         wk_sb[:, 96 * h:96 * h + 96],
                                         x_sb[:, cs], start=True, stop=True)
                        nc.vector.tensor_copy(k_rep[h][:, cs], pk[0:32 * REP, :])
                    pv_ = ppsum.tile([128, 512], f32, tag="proj")
                    nc.tensor.matmul(pv_[0:64, :], wv_sb[:, :], x_sb[:, cs],
                                     start=True, stop=True)
                    for h in range(2):
                        nc.vector.tensor_copy(v_sb[h][:, cs],
                                              pv_[32 * h:32 * h + 32, :])

                # v^T per j-tile via TensorE transpose ([32,128] -> [128,32])
                for h in range(2):
                    for jt in range(JT):
                        tp = tpsum.tile([128, 32], f32, tag="tp")
                        nc.tensor.transpose(tp[:, :],
                                            v_sb[h][:, 128 * jt:128 * (jt + 1)],
                                            ident[:, :])
                        nc.vector.tensor_copy(vt1[h][:, 33 * jt:33 * jt + 32],
                                              tp[:, :])

            with tc.tile_pool(name="spsum", bufs=2, space="PSUM") as spsum, \
                 tc.tile_pool(name="accp", bufs=1, space="PSUM") as accp, \
                 tc.tile_pool(name="rbp", bufs=1, space="PSUM") as rbp, \
                 tc.tile_pool(name="ptp", bufs=3) as ptp, \
                 tc.tile_pool(name="rsp", bufs=2) as rsp:
                for h in range(2):
                    for ib in range(NCH):  # 512-wide i blocks
                        ibs = slice(512 * ib, 512 * (ib + 1))
                        acc = accp.tile([33, 512], f32, tag="acc")
                        first = True
                        for grp in groups:
                            gs = len(grp)
                            sp = spsum.tile([128, 512 * REP], f32, tag="sp")
                            for r, jt in enumerate(grp):
                                # strip r: lhsT/rhs live at partitions 32r..32r+32
                                nc.tensor.matmul(
                                    sp[:, 512 * r:512 * (r + 1)],
                                    k_rep[h][32 * r:32 * r + 32,
                                             128 * jt:128 * (jt + 1)],
                                    q_rep[h][32 * r:32 * r + 32, ibs],
                                    start=True, stop=True)
                            pt = ptp.tile([128, 512 * REP], f32, tag="pt")
                            nc.scalar.activation(
                                pt[:, 0:512 * gs], sp[:, 0:512 * gs],
                                mybir.ActivationFunctionType.Exp, scale=SCALE)
                            for r, jt in enumerate(grp):
                                last = jt == JT - 1
                                nc.tensor.matmul(
                                    acc[:, :],
                                    vt1[h][:, 33 * jt:33 * (jt + 1)],
                                    pt[:, 512 * r:512 * (r + 1)],
                                    start=first, stop=last,
                                    skip_group_check=True)
                                first = False
                        # normalize: o = acc[0:32] * (1/rowsum) broadcast
                        rs = rsp.tile([1, 512], f32, tag="rs")
                        nc.vector.reciprocal(rs[:, :], acc[32:33, :])
                        rb = rbp.tile([32, 512], f32, tag="rb")
                        nc.tensor.matmul(rb[:, :], ones1[:, :], rs[:, :],
                                         start=True, stop=True)
                        r32 = rsp.tile([32, 512], f32, tag="r32")
                        nc.vector.tensor_copy(r32[:, :], rb[:, :])
                        nc.vector.tensor_mul(o_sb[32 * h:32 * h + 32, ibs],
                                             acc[0:32, :], r32[:, :])

            with tc.tile_pool(name="opsum", bufs=2, space="PSUM") as opsum:
                for ch in range(NCH):
                    cs = slice(512 * ch, 512 * (ch + 1))
                    po = opsum.tile([128, 512], f32, tag="po")
                    nc.tensor.matmul(po[:, :], wo_sb[:, :], o_sb[:, cs],
                                     start=True, stop=True)
                    nc.vector.tensor_copy(out_sb[:, cs], po[:, :])
            nc.gpsimd.dma_start(out_ext[:, :], out_sb[:, :])
    nc.compile()
    return nc


def _make_in_maps(x, w_qkv, w_out):
    in_maps = []
    for core in range(NCORES):
        b = core // 2
        g = core % 2
        heads = [2 * g, 2 * g + 1]
        wq_cols, wk_cols = [], []
        for h in heads:
            wq_h = w_qkv[32 * h:32 * h + 32, :]            # [32, 128]
            wk_h = w_qkv[128 + 32 * h:128 + 32 * h + 32, :]
            wq_cols.append(np.tile(wq_h.T, (1, REP)))      # [128, 96]
            wk_cols.append(np.tile(wk_h.T, (1, REP)))
        wv_cols = [w_qkv[256 + 32 * h:256 + 32 * h + 32, :].T for h in heads]
        in_maps.append({
            "x": np.ascontiguousarray(x[b].reshape(C, N), dtype=np.float32),
            "wq": np.ascontiguousarray(np.concatenate(wq_cols, axis=1), dtype=np.float32),
            "wk": np.ascontiguousarray(np.concatenate(wk_cols, axis=1), dtype=np.float32),
            "wv": np.ascontiguousarray(np.concatenate(wv_cols, axis=1), dtype=np.float32),
            "wo": np.ascontiguousarray(w_out[:, 64 * g:64 * g + 64].T, dtype=np.float32),
        })
    return in_maps


def _run(x, w_qkv, w_out, trace=False):
    from concourse.bass_utils import run_bass_kernel_spmd
    if "nc" not in _CACHE:
        _CACHE["nc"] = _build_bass()
    nc = _CACHE["nc"]
    in_maps = _make_in_maps(x, w_qkv, w_out)
    res = run_bass_kernel_spmd(nc, in_maps, core_ids=list(range(NCORES)),
                               trace=trace)
    return res


def kernel(x, w_qkv, w_out, b_out):
    x = np.asarray(x, dtype=np.float32)
    w_qkv = np.asarray(w_qkv, dtype=np.float32)
    w_out = np.asarray(w_out, dtype=np.float32)
    b_out = np.asarray(b_out, dtype=np.float32)
    res = _run(x, w_qkv, w_out, trace=False)
    outs = [res.results[c]["out"] for c in range(NCORES)]
    full = np.empty((B, C, H, W), dtype=np.float32)
    for b in range(B):
        s = outs[2 * b] + outs[2 * b + 1] + b_out[:, None]
        full[b] = s.reshape(C, H, W)
    return full
